# revision 1
# baseline (speedup 1.0000x reference)
"""Tied-row (MSA) attention on 8 Trainium2 NeuronCores.

Reference computation (B=128, n=512, dim=256, h=8, dh=64,
r=tie_attn_dim=64, b=B//r=2):
    q = x @ Wq ; k,v = split(x @ Wkv)
    dots[b,h,i,j] = sum_{r,d} q[b,r,h,i,d] k[b,r,h,j,d] * scale
    attn = softmax_j(dots)
    out[b,r,h,i,d] = sum_j attn[b,h,i,j] v[b,r,h,j,d]
    y = out @ Wo + bo

Sharding: 8 cores = b(2) x head-pairs(4).  Each core owns one batch
element and 2 of the 8 heads and produces the partial
    y_part = out[:, :, own 2 heads, :] @ Wo[own 128 rows, :]
The host sums the 4 partials per b and adds bo (the head reduction of
the output projection commutes with the sum).

Per-core device kernel (shapes hardcoded):
  inputs : xT [64, 256, 512] f16   (x[b] transposed to [r, c, n])
           wq,wk,wv [256, 128] f16 (wq pre-scaled by dh^-.5 * r^-.5)
           wo [128, 256] f16
  output : y  [64, 512, 256] f32   (partial)

  Phase 1 + dots wave A fused (r-loop): qT_r/kT_r projections -> PSUM ->
          resident fp16 q_all/k_all [128=(2h x 64d), r, n]; dots for
          i-tiles 0,1 accumulate in 4 banks one iteration behind the
          copies (PSUM: 2 q + 2 k + 4 dots = 8 banks; the two heads'
          K=64 dots matmuls auto row-tile via base_partition 0/64 and
          run concurrently).  Wave A softmax inside this PSUM scope.
  Wave B: dots i-tiles 2,3 accumulate from resident q/k; attn tiles are
          transposed to attnT fp16 by single xbar DMA transposes
          (out[j, jc, i] = attn[i, jc*128+j]), overlapping the wave.
  Phase 3 (r-loop, 2-deep SW pipeline A=v, B=out, C=y): reload xT_r,
          v_r = xT_r.T @ wv, out_rT[hd, i] over j-chunks (lhsT = v f16,
          rhs = attnT f16; jc-outer/h-inner emission so the two heads'
          M=64 matmuls col-tile concurrently, skip_group_check for the
          interleaved PSUM groups), y_r[i, e] = out_rT.T @ wo, 8-row
          blocked DMA out on the ACT queue.

  Built with bacc.Bacc(): its compile() pass legalizes Tile's sync for
  this walrus (which caps sync waits per instruction); callers must
  finalize() the program before running (see _get_program).
"""

import os
import sys

for _p in ("/opt/trn_rl_repo", "/root/.axon_site/_ro/trn_rl_repo"):
    if os.path.isdir(_p) and _p not in sys.path:
        sys.path.insert(0, _p)

import numpy as np

R = 64          # tie dim (MSA rows per batch element)
RB = 8          # rows per DMA block
N = 512         # sequence length
C = 256         # model dim
HP = 128        # head-pair width: 2 heads x 64
E = 256         # output dim
NCORES = 8

_CACHE = {}


def build_program(phases=(1, 2, 3)):
    import concourse.bacc as bacc
    from concourse import mybir
    from concourse.tile import TileContext
    from contextlib import ExitStack

    f32 = mybir.dt.float32
    f16 = mybir.dt.float16

    # Bacc (not bass.Bass): its compile() pass legalizes sync for walrus --
    # moves matmul waits onto LDWEIGHTS and lowers multi-wait instructions
    # to event semaphores.  Raw Tile output violates walrus's per-struct
    # sync-wait limits.
    nc = bacc.Bacc()
    xT = nc.declare_dram_parameter("xT", [R, C, N], f16, isOutput=False)
    wq = nc.declare_dram_parameter("wq", [C, HP], f16, isOutput=False)
    wk = nc.declare_dram_parameter("wk", [C, HP], f16, isOutput=False)
    wv = nc.declare_dram_parameter("wv", [C, HP], f16, isOutput=False)
    wo = nc.declare_dram_parameter("wo", [HP, E], f16, isOutput=False)
    y = nc.declare_dram_parameter("y", [R, N, E], f32, isOutput=True)

    # xT block rb viewed as [p, r_in_block, c_chunk, n]
    xT_blk = xT.rearrange("(rb r) (cc p) n -> rb p r cc n", r=RB, p=128)
    # y block rb viewed as [p, r_in_block, i_tile, e]
    y_blk = y.rearrange("(rb r) (t p) e -> rb p r t e", r=RB, p=128)

    with TileContext(nc) as tc, ExitStack() as ctx:
        singles = ctx.enter_context(tc.tile_pool(name="singles", bufs=1))
        sm = ctx.enter_context(tc.tile_pool(name="sm", bufs=4))
        attnp = ctx.enter_context(tc.tile_pool(name="attnp", bufs=4))
        attntp = ctx.enter_context(tc.tile_pool(name="attntp", bufs=2))

        # weights: [256, X] -> sbuf [128, 2, X] (c-chunk on free axis)
        wq_sb = singles.tile([128, 2, HP], f16)
        wk_sb = singles.tile([128, 2, HP], f16)
        wv_sb = singles.tile([128, 2, HP], f16)
        wo_sb = singles.tile([128, E], f16)
        for cc in range(2):
            nc.gpsimd.dma_start(out=wq_sb[:, cc, :], in_=wq[cc * 128:(cc + 1) * 128, :])
            nc.gpsimd.dma_start(out=wk_sb[:, cc, :], in_=wk[cc * 128:(cc + 1) * 128, :])
            nc.gpsimd.dma_start(out=wv_sb[:, cc, :], in_=wv[cc * 128:(cc + 1) * 128, :])
        nc.gpsimd.dma_start(out=wo_sb, in_=wo[:, :])

        # attnT survives into phase 3: kernel-scoped pool
        attnT = [attntp.tile([128, 4, N], f16, tag="attnT", name=f"attnT_{h}")
                 for h in range(2)]

        def softmax(dots_hit, h, it):
            """dots PSUM tile -> normalized f16 attn SBUF tile.

            No max-subtraction: dots = q k^T with the 1/(sqrt(dh) sqrt(r))
            scale folded into Wq, so entries are ~N(0,1) and exp cannot
            overflow fp32/fp16.  This keeps ACT as the only dots reader
            (walrus allows at most 2 sync waits per instruction)."""
            ssum = sm.tile([128, 1], f32, tag="ssum", bufs=8)
            rinv = sm.tile([128, 1], f32, tag="rinv", bufs=8)
            attn = attnp.tile([128, N], f16, tag="attn", bufs=8,
                              name=f"attn_{h}_{it}")
            nc.scalar.activation(
                out=attn, in_=dots_hit,
                func=mybir.ActivationFunctionType.Exp,
                accum_out=ssum)
            nc.vector.reciprocal(rinv, ssum)
            nc.vector.tensor_scalar_mul(attn, attn, rinv)
            return attn

        def transpose_attn(ps_pool, attn, h, it):
            # one f16 xbar DMA transpose, SBUF -> SBUF: out[j, jc, i] =
            # attn[i, jc*128 + j]; no PE/PSUM involvement
            nc.sync.dma_start_transpose(
                out=attnT[h][:, :, it * 128:(it + 1) * 128], in_=attn)

        xpool = ctx.enter_context(tc.tile_pool(name="xpool", bufs=2))

        # resident fp16 qT/kT live only through phases 1-2
        with tc.tile_pool(name="resid", bufs=1) as resid:
            q_all = resid.tile([128, R, N], f16)
            k_all = resid.tile([128, R, N], f16)

            def dots_wave(dots_tiles, r, its):
                for it in its:
                    for h in range(2):
                        hs = slice(h * 64, (h + 1) * 64)
                        nc.tensor.matmul(
                            dots_tiles[h][it % 2],
                            lhsT=q_all[hs, r, it * 128:(it + 1) * 128],
                            rhs=k_all[hs, r, :],
                            start=(r == 0), stop=(r == R - 1))

            # -------- Phase 1 + dots wave A (i-tiles 0,1) fused --------
            attnA = {}
            with tc.tile_pool(name="ps1", space="PSUM", bufs=2) as ps1:
                dotsA = [[ps1.tile([128, N], f32, tag="dots", bufs=4,
                                   name=f"dotsA_{h}_{it}")
                          for it in range(2)] for h in range(2)]
                n_r = R if 1 in phases else 0
                for r in range(n_r + 1):
                    if r < n_r:
                        rb, ri = divmod(r, RB)
                        if ri == 0:
                            x_sb = xpool.tile([128, RB, 2, N], f16, tag="x",
                                              name=f"x1_{rb}")
                            nc.sync.dma_start(out=x_sb, in_=xT_blk[rb])
                        q_ps = ps1.tile([128, N], f32, tag="q")
                        k_ps = ps1.tile([128, N], f32, tag="k")
                        for cc in range(2):
                            nc.tensor.matmul(q_ps, lhsT=wq_sb[:, cc, :],
                                             rhs=x_sb[:, ri, cc, :],
                                             start=(cc == 0), stop=(cc == 1))
                        for cc in range(2):
                            nc.tensor.matmul(k_ps, lhsT=wk_sb[:, cc, :],
                                             rhs=x_sb[:, ri, cc, :],
                                             start=(cc == 0), stop=(cc == 1))
                        nc.vector.tensor_copy(q_all[:, r, :], q_ps)
                        nc.scalar.copy(k_all[:, r, :], k_ps)
                    if 0 <= r - 1 < n_r and 2 in phases:
                        dots_wave(dotsA, r - 1, (0, 1))
                # wave A softmax consumes the dots PSUM inside this scope
                for h in range(2 if 2 in phases else 0):
                    for it in range(2):
                        attnA[(h, it)] = softmax(dotsA[h][it], h, it)

            # -------- dots wave B + all transposes --------
            with tc.tile_pool(name="ps2", space="PSUM", bufs=2) as ps2:
                dotsB = [[ps2.tile([128, N], f32, tag="dots", bufs=4,
                                   name=f"dotsB_{h}_{it}")
                          for it in range(2)] for h in range(2)]
                for r in range(R if 2 in phases else 0):
                    dots_wave(dotsB, r, (2, 3))
                # wave A transposes overlap wave B's accumulation (PE is
                # in-order, but DVE copies and softmaxes interleave)
                for (h, it), attn in attnA.items():
                    transpose_attn(ps2, attn, h, it)
                for h in range(2 if 2 in phases else 0):
                    for it in (2, 3):
                        attn = softmax(dotsB[h][it % 2], h, it)
                        transpose_attn(ps2, attn, h, it)

        # ---------------- Phase 3: v, out, y (2-deep SW pipeline) ----------------
        with tc.tile_pool(name="ps3", space="PSUM", bufs=2) as ps3, \
             tc.tile_pool(name="vpool", bufs=4) as vpool, \
             tc.tile_pool(name="outp", bufs=4) as outp, \
             tc.tile_pool(name="ypool", bufs=2) as ypool:
            n_r = R if 3 in phases else 0
            v_sbs = {}
            out_sbs = {}
            y_sbs = {}

            def stage_a(r, x_sb, ri):
                v_ps = ps3.tile([128, 4, 128], f32, tag="v", name=f"v_ps_{r}")
                for jt in range(4):
                    for cc in range(2):
                        nc.tensor.matmul(
                            v_ps[:, jt, :],
                            lhsT=x_sb[:, ri, cc, jt * 128:(jt + 1) * 128],
                            rhs=wv_sb[:, cc, :],
                            start=(cc == 0), stop=(cc == 1))
                v_sb = vpool.tile([128, 4, 128], f16, tag="vsb", name=f"v_sb_{r}")
                nc.scalar.copy(v_sb, v_ps)
                v_sbs[r] = v_sb

            def stage_b(r):
                v_sb = v_sbs.pop(r)
                out_ps = ps3.tile([128, N], f32, tag="out", name=f"out_ps_{r}")
                # jc-outer / h-inner: adjacent matmuls hit different PE col
                # groups (out partitions 0-63 / 64-127) and run concurrently
                for jc in range(4):
                    for h in range(2):
                        hs = slice(h * 64, (h + 1) * 64)
                        nc.tensor.matmul(
                            out_ps[hs, :],
                            lhsT=v_sb[:, jc, hs],
                            rhs=attnT[h][:, jc, :],
                            start=(jc == 0), stop=(jc == 3),
                            skip_group_check=True)
                out_sb = outp.tile([128, N], f16, tag="outsb", name=f"out_sb_{r}")
                nc.vector.tensor_copy(out_sb, out_ps)
                out_sbs[r] = out_sb

            def stage_c(r):
                out_sb = out_sbs.pop(r)
                y_ps = ps3.tile([128, 4, E], f32, tag="y", name=f"y_ps_{r}")
                for it in range(4):
                    nc.tensor.matmul(
                        y_ps[:, it, :],
                        lhsT=out_sb[:, it * 128:(it + 1) * 128],
                        rhs=wo_sb,
                        start=True, stop=True)
                rb, ri = divmod(r, RB)
                if ri == 0:
                    y_sbs[rb] = ypool.tile([128, RB, 4, E], f32, tag="ysb",
                                           name=f"y_sb_{rb}")
                y_sb = y_sbs[rb]
                nc.vector.tensor_copy(y_sb[:, ri, 0:2, :], y_ps[:, 0:2, :])
                nc.scalar.copy(y_sb[:, ri, 2:4, :], y_ps[:, 2:4, :])
                if ri == RB - 1:
                    nc.scalar.dma_start(out=y_blk[rb], in_=y_sbs.pop(rb))

            x_tiles = {}
            for r in range(n_r + 2):
                if r < n_r:
                    rb, ri = divmod(r, RB)
                    if ri == 0:
                        x_tiles[rb] = xpool.tile([128, RB, 2, N], f16, tag="x",
                                                 name=f"x3_{rb}")
                        nc.sync.dma_start(out=x_tiles[rb], in_=xT_blk[rb])
                    stage_a(r, x_tiles[rb], ri)
                if 0 <= r - 1 < n_r:
                    stage_b(r - 1)
                if 0 <= r - 2 < n_r:
                    stage_c(r - 2)

    return nc


def _get_program():
    if "nc" not in _CACHE:
        nc = build_program()
        nc.finalize()
        _CACHE["nc"] = nc
    return _CACHE["nc"]


def make_in_maps(x, Wq, Wkv, Wo):
    """Host-side sharding: core = bi*4 + hpi."""
    scale = (64.0 ** -0.5) * (64.0 ** -0.5)
    x = np.asarray(x, np.float32)
    Wq = np.asarray(Wq, np.float32) * np.float32(scale)
    Wkv = np.asarray(Wkv, np.float32)
    Wo = np.asarray(Wo, np.float32)
    b = x.shape[0] // R
    xT = np.ascontiguousarray(
        x.reshape(b, R, N, C).transpose(0, 1, 3, 2)).astype(np.float16)
    in_maps = []
    for core in range(NCORES):
        bi, hpi = divmod(core, 4)
        cols = slice(hpi * HP, (hpi + 1) * HP)
        in_maps.append({
            "xT": xT[bi],
            "wq": np.ascontiguousarray(Wq[:, cols]).astype(np.float16),
            "wk": np.ascontiguousarray(Wkv[:, cols]).astype(np.float16),
            "wv": np.ascontiguousarray(
                Wkv[:, 512 + hpi * HP: 512 + (hpi + 1) * HP]).astype(np.float16),
            "wo": np.ascontiguousarray(Wo[cols, :]).astype(np.float16),
        })
    return in_maps


def combine_outputs(ys, bo):
    """ys: list of 8 [R, N, E] partials in core order; returns [B, n, dim]."""
    y0 = ys[0] + ys[1] + ys[2] + ys[3]
    y1 = ys[4] + ys[5] + ys[6] + ys[7]
    y = np.concatenate([y0, y1], axis=0).reshape(2 * R, N, E)
    return (y + np.asarray(bo, np.float32)).astype(np.float32)


def kernel(x, Wq, Wkv, Wo, bo, tie_attn_dim):
    assert int(tie_attn_dim) == R, f"hardcoded for tie_attn_dim={R}"
    from concourse.bass_utils import run_bass_kernel_spmd

    nc = _get_program()
    in_maps = make_in_maps(x, Wq, Wkv, Wo)
    res = run_bass_kernel_spmd(nc, in_maps, list(range(NCORES)))
    ys = [np.asarray(res.results[c]["y"], np.float32) for c in range(NCORES)]
    return combine_outputs(ys, bo)



# revision 36
# speedup vs baseline: 1.4193x; 1.4193x over previous
"""Tied-row (MSA) attention on 8 Trainium2 NeuronCores.

Reference computation (B=128, n=512, dim=256, h=8, dh=64,
r=tie_attn_dim=64, b=B//r=2):
    q = x @ Wq ; k,v = split(x @ Wkv)
    dots[b,h,i,j] = sum_{r,d} q[b,r,h,i,d] k[b,r,h,j,d] * scale
    attn = softmax_j(dots)
    out[b,r,h,i,d] = sum_j attn[b,h,i,j] v[b,r,h,j,d]
    y = out @ Wo + bo

Sharding: 8 cores = b(2) x head-pairs(4).  Each core owns one batch
element and 2 of the 8 heads and produces the partial
    y_part = out[:, :, own 2 heads, :] @ Wo[own 128 rows, :]
The host sums the 4 partials per b and adds bo.

Per-core kernel, restructured so every matmul streams with a full
128-deep contraction and full output partitions (matmul cost is
output-free-size per instruction; K and partition count are free):

  Phase 1 (r-loop): q_r/k_r projections [hd, n] -> PSUM -> fp16 staging
          blocks of G rows; per block, 8 partition-remap DMAs scatter
          (head h, row-parity rp) 64-partition slabs into
              q_lay/k_lay [128=(rp,d), h, rpair, n]
          so each row PAIR forms one 128-deep contraction slice.  v for
          the first NV rows is projected ([j, (rp,d)] pair layout) and
          kept resident.  dots wave A (i-tiles 0,1; 4 PSUM banks)
          accumulates over r-pairs with K=128 (32 matmuls per (h,it)
          instead of 64 K=64), paced one block behind the remaps.
  Wave B: dots i-tiles 2,3 from resident q_lay/k_lay; softmax (ACT exp
          with accum, DVE reciprocal+scale; no max-subtraction -- the
          scale folded into Wq keeps dots ~N(0,1)); attn transposed to
          attnT[h] [j, jc, i] fp16 by xbar DMA transposes.  Phase-3 x
          reload prefetches during this PE-only window.
  Phase 3 (r-pair loop): v for r>=NV from reloaded x; out matmuls use
          lhsT=attnT slices (M=i-tile 128), rhs=v-pair [j, (rp,d)] ->
          PSUM [i, (it,h,rp,d)] -- K=128, M=128: half the old cost.
          Per GO-row block one xbar DMA transpose turns fp16 staging
          [i, g, (it,h,d)] into outT [hd, (g,it), i]; y_r = outT.T @ wo
          -> fp16 y, written per RBY-row block.  y-stage lags YLAG pairs
          behind out-stage (the swap DMA must land), emitted split
          around the v/out work to keep PE fed.

  Built with bacc.Bacc(): its compile() pass legalizes Tile's sync for
  walrus; callers must finalize() before running (see _get_program).
"""

import os
import sys

for _p in ("/opt/trn_rl_repo", "/root/.axon_site/_ro/trn_rl_repo"):
    if os.path.isdir(_p) and _p not in sys.path:
        sys.path.insert(0, _p)

import numpy as np

R = 64          # tie dim (MSA rows per batch element)
NP = R // 2     # row pairs
N = 512         # sequence length
C = 256         # model dim
HP = 128        # head-pair width: 2 heads x 64
E = 256         # output dim
NCORES = 8

G = 8           # staging block rows (phase 1 remap granularity)
RB1 = 4         # x-load block rows (phase 1)
RB3 = 4         # x-load block rows (phase 3)
GO = 2          # out-staging block pairs (phase 3 y-remap granularity)
RBY = 2         # y-store block rows
YLAG = 3        # pairs of lag between out-stage and y-stage
VLEAD = 2       # pairs of lead for the v load-transpose

_CACHE = {}


def build_program(phases=(1, 2, 3)):
    import concourse.bacc as bacc
    from concourse import mybir
    from concourse.tile import TileContext
    from contextlib import ExitStack

    f32 = mybir.dt.float32
    f16 = mybir.dt.float16

    nc = bacc.Bacc()
    xT = nc.declare_dram_parameter("xT", [R, C, N], f16, isOutput=False)
    wq = nc.declare_dram_parameter("wq", [C, HP], f16, isOutput=False)
    wk = nc.declare_dram_parameter("wk", [C, HP], f16, isOutput=False)
    wv = nc.declare_dram_parameter("wv", [C, HP], f16, isOutput=False)
    wo = nc.declare_dram_parameter("wo", [HP, E], f16, isOutput=False)
    y = nc.declare_dram_parameter("y", [R, 2, 128, N], f16, isOutput=True)

    # xT block rb viewed as [p, r_in_block, c_chunk, n]
    x_blk1 = xT.rearrange("(rb r) (cc p) n -> rb p r cc n", r=RB1, p=128)
    x_blk3 = xT.rearrange("(rb r) (cc p) n -> rb p r cc n", r=RB3, p=128)
    # y block rb viewed as [p=e128, r_in_block, et, i]
    y_blk = y.rearrange("(rb r) et p n -> rb p r et n", r=RBY)

    do12 = 1 in phases
    do2 = 2 in phases
    do3 = 3 in phases

    with TileContext(nc) as tc, ExitStack() as ctx:
        singles = ctx.enter_context(tc.tile_pool(name="singles", bufs=1))
        sm = ctx.enter_context(tc.tile_pool(name="sm", bufs=4))
        attnp = ctx.enter_context(tc.tile_pool(name="attnp", bufs=4))
        attntp = ctx.enter_context(tc.tile_pool(name="attntp", bufs=2))
        xpool3 = ctx.enter_context(tc.tile_pool(name="xpool3", bufs=2))

        # weights: [256, X] -> sbuf [128, 2, X] (c-chunk on free axis)
        wq_sb = singles.tile([128, 2, HP], f16)
        wk_sb = singles.tile([128, 2, HP], f16)
        wv_sb = singles.tile([128, 2, HP], f16)
        wo_sb = singles.tile([128, E], f16)
        for cc in range(2):
            nc.sync.dma_start(out=wq_sb[:, cc, :], in_=wq[cc * 128:(cc + 1) * 128, :])
            nc.sync.dma_start(out=wk_sb[:, cc, :], in_=wk[cc * 128:(cc + 1) * 128, :])
            nc.sync.dma_start(out=wv_sb[:, cc, :], in_=wv[cc * 128:(cc + 1) * 128, :])
        nc.sync.dma_start(out=wo_sb, in_=wo[:, :])

        # attnT survives into phase 3
        attnT = [attntp.tile([128, 4, N], f16, tag="attnT", name=f"attnT_{h}")
                 for h in range(2)]

        def softmax(dots_hit, h, it):
            """dots PSUM tile -> normalized f16 attn SBUF tile.

            No max-subtraction: dots = q k^T with the 1/(sqrt(dh) sqrt(r))
            scale folded into Wq, so entries are ~N(0,1) and exp cannot
            overflow.  Keeps ACT as the only dots reader."""
            ssum = sm.tile([128, 1], f32, tag="ssum", bufs=8)
            rinv = sm.tile([128, 1], f32, tag="rinv", bufs=8)
            attn = attnp.tile([128, N], f16, tag="attn", bufs=4,
                              name=f"attn_{h}_{it}")
            nc.scalar.activation(
                out=attn, in_=dots_hit,
                func=mybir.ActivationFunctionType.Exp,
                accum_out=ssum)
            nc.vector.reciprocal(rinv, ssum)
            nc.vector.tensor_scalar_mul(attn, attn, rinv)
            return attn

        def transpose_attn(attn, h, it):
            # f16 xbar DMA transpose: attnT[h][j, jc, i] = attn[i, jc*128+j]
            nc.sync.dma_start_transpose(
                out=attnT[h][:, :, it * 128:(it + 1) * 128], in_=attn)

        # ---------------- Phases 1-2: q/k -> q_lay/k_lay -> dots ----------------
        attnA = {}
        with tc.tile_pool(name="laypool", bufs=1) as laypool:
            # [ (rp,d), h, pair, n ]
            q_lay = laypool.tile([128, 2, NP, N], f16)
            k_lay = laypool.tile([128, 2, NP, N], f16)

            def dots_pair(dots_tiles, pair, its):
                for h in range(2):
                    for it in its:
                        nc.tensor.matmul(
                            dots_tiles[h][it % 2],
                            lhsT=q_lay[:, h, pair, it * 128:(it + 1) * 128],
                            rhs=k_lay[:, h, pair, :],
                            start=(pair == 0), stop=(pair == NP - 1))

            def remap(stage, lay, blk):
                # even-row h1 halves are parked at lay[64:128, h=0]; move
                # them home, then refill the park with the staged odd-row
                # h0 halves (Tile orders the WAR hazard)
                ps = slice(blk * (G // 2), (blk + 1) * (G // 2))
                nc.sync.dma_start(out=lay[0:64, 1, ps, :],
                                  in_=lay[64:128, 0, ps, :])
                nc.sync.dma_start(out=lay[64:128, 0, ps, :],
                                  in_=stage[0:64, :, :])

            # -------- Phase 1 (+ dots wave A paced one block behind) --------
            with tc.tile_pool(name="ps1", space="PSUM", bufs=1) as ps1, \
                 tc.tile_pool(name="stgq", bufs=2) as stgq, \
                 tc.tile_pool(name="stgk", bufs=2) as stgk, \
                 tc.tile_pool(name="xpool1", bufs=3) as xpool1:
                dotsA = [[ps1.tile([128, N], f32, tag=f"dotsA_{h}_{it}",
                                   name=f"dotsA_{h}_{it}")
                          for it in range(2)] for h in range(2)]
                n_r = R if do12 else 0
                qs = ks = None
                for r in range(n_r):
                    rb, ri = divmod(r, RB1)
                    if ri == 0:
                        x_sb = xpool1.tile([128, RB1, 2, N], f16, tag="x",
                                           name=f"x1_{rb}")
                        # split loads: readers unblock progressively
                        nsp = RB1 if rb == 0 else 2
                        eng = nc.gpsimd
                        for hb in range(nsp):
                            hs = slice(hb * (RB1 // nsp),
                                       (hb + 1) * (RB1 // nsp))
                            eng.dma_start(out=x_sb[:, hs, :, :],
                                          in_=x_blk1[rb, :, hs, :, :])
                    gb, (gp, rp) = r // G, divmod(r % G, 2)
                    if r % G == 0:
                        qs = stgq.tile([128, G // 2, N], f16, tag="qs",
                                       name=f"qs_{gb}")
                        ks = stgk.tile([128, G // 2, N], f16, tag="ks",
                                       name=f"ks_{gb}")
                    q_ps = ps1.tile([128, N], f32, tag="q", bufs=2)
                    k_ps = ps1.tile([128, N], f32, tag="k", bufs=2)
                    for cc in range(2):
                        nc.tensor.matmul(q_ps, lhsT=wq_sb[:, cc, :],
                                         rhs=x_sb[:, ri, cc, :],
                                         start=(cc == 0), stop=(cc == 1))
                    for cc in range(2):
                        nc.tensor.matmul(k_ps, lhsT=wk_sb[:, cc, :],
                                         rhs=x_sb[:, ri, cc, :],
                                         start=(cc == 0), stop=(cc == 1))
                    nc.vector.tensor_copy(qs[:, gp, rp, :], q_ps)
                    nc.scalar.copy(ks[:, gp, rp, :], k_ps)
                    if r < NV:
                        project_v(r, x_sb, ri, v_res[:, r // 2, :, :, :])
                    if r % G == G - 1:
                        remap(qs, q_lay, gb)
                        remap(ks, k_lay, gb)
                    # dots wave A: pairs of block gb-2, one per 2 rows
                    # (two blocks of lag so the remap DMA has landed)
                    if do2 and r >= 2 * G and r % 2 == 1:
                        pair = (gb - 2) * (G // 2) + r % G // 2
                        dots_pair(dotsA, pair, (0, 1))
                if do2:
                    # leftover wave-A pairs, tile-major with fused softmax
                    # so each tile's exp starts as soon as it completes
                    first = NP - 2 * (G // 2) if do12 else 0
                    for h in range(2):
                        for it in range(2):
                            for pair in range(first, NP):
                                nc.tensor.matmul(
                                    dotsA[h][it],
                                    lhsT=q_lay[:, h, pair,
                                               it * 128:(it + 1) * 128],
                                    rhs=k_lay[:, h, pair, :],
                                    start=(pair == 0),
                                    stop=(pair == NP - 1))
                            attnA[(h, it)] = softmax(dotsA[h][it], h, it)

            # -------- dots wave B + transposes + x-reload prefetch --------
            with tc.tile_pool(name="xpool3", bufs=4) as xpool3, \
                 tc.tile_pool(name="vpool", bufs=3) as vpool, \
                 tc.tile_pool(name="outsp", bufs=2) as outsp, \
                 tc.tile_pool(name="outtp", bufs=2) as outtp, \
                 tc.tile_pool(name="ypool", bufs=4) as ypool:
                with tc.tile_pool(name="ps2", space="PSUM", bufs=1) as ps2:
                    if do2:
                        dotsB = [[ps2.tile([128, N], f32, tag=f"dotsB_{h}_{it}",
                                           name=f"dotsB_{h}_{it}")
                                  for it in range(2)] for h in range(2)]
                        for pair in range(NP - 4):
                            dots_pair(dotsB, pair, (2, 3))
                        # wave A transposes overlap wave B accumulation
                        for (h, it), attn in attnA.items():
                            transpose_attn(attn, h, it)
                        # tail tile-major: each tile's softmax fires early
                        for h in range(2):
                            for it in (2, 3):
                                for pair in range(NP - 4, NP):
                                    nc.tensor.matmul(
                                        dotsB[h][it % 2],
                                        lhsT=q_lay[:, h, pair,
                                                   it * 128:(it + 1) * 128],
                                        rhs=k_lay[:, h, pair, :],
                                        start=(pair == 0),
                                        stop=(pair == NP - 1))
                                attn = softmax(dotsB[h][it % 2], h, it)
                                transpose_attn(attn, h, it)

                # ---------------- Phase 3 ----------------
                with tc.tile_pool(name="ps3", space="PSUM", bufs=1) as ps3:
                    n_p3 = NP if do3 else 0
                    x_tiles = {}
                    vcur = {}
                    ostg = {}
                    otsp = {}
                    y_sbs = {}

                    def stage_v(pair):
                        r0 = pair * 2
                        if r0 >= NV:
                            v_pair = vpool.tile([128, 4, 2, 128], f16,
                                                tag="vp", name=f"vp_{pair}")
                            for rp in range(2):
                                r = r0 + rp
                                rb, ri = divmod(r - NV, RB3)
                                if ri == 0:
                                    xt = xpool3.tile([128, RB3, 2, N], f16,
                                                     tag="x3", name=f"x3_{rb}")
                                    nc.sync.dma_start(
                                        out=xt, in_=x_blk3[rb + NV // RB3])
                                    x_tiles[rb] = xt
                                project_v(r, x_tiles[rb], ri, v_pair)
                            vcur[pair] = v_pair
                        else:
                            vcur[pair] = v_res[:, pair, :, :, :]

                    def stage_out(pair):
                        """out matmuls for one r-pair + fp16 staging copies."""
                        out_ps = ps3.tile([128, 4, 2, 2, 64], f32, tag="out",
                                          bufs=2, name=f"out_ps_{pair}")
                        v_pair = vcur.pop(pair)
                        for it in range(4):
                            for h in range(2):
                                for jc in range(4):
                                    nc.tensor.matmul(
                                        out_ps[:, it, h, :, :],
                                        lhsT=attnT[h][:, jc,
                                                      it * 128:(it + 1) * 128],
                                        rhs=v_pair[:, jc, h, :],
                                        start=(jc == 0), stop=(jc == 3))
                        ob, og = divmod(pair * 2, GO)
                        if og == 0:
                            ostg[ob] = outsp.tile([128, GO, 4, 2, 64], f16,
                                                  tag="os", name=f"os_{ob}")
                        # split the pair back into per-row (it,h,d) staging
                        for rp in range(2):
                            nc.vector.tensor_copy(
                                ostg[ob][:, og + rp, :, :, :],
                                out_ps[:, :, :, rp, :])
                        if og == GO - 2:
                            ot = outtp.tile([128, 4 * GO, 128], f16, tag="ot",
                                            name=f"ot_{ob}")
                            nc.sync.dma_start_transpose(out=ot, in_=ostg.pop(ob))
                            otsp[ob] = ot

                    def stage_y(r):
                        """y_r = outT.T @ wo -> fp16 y_sb -> DMA per block."""
                        ob, og = divmod(r, GO)
                        ot = otsp[ob]
                        y_ps = ps3.tile([128, 4, E], f32, tag="y", bufs=1,
                                        name=f"y_ps_{r}")
                        for it in range(4):
                            nc.tensor.matmul(
                                y_ps[:, it, :],
                                lhsT=ot[:, og * 4 + it, :],
                                rhs=wo_sb,
                                start=True, stop=True)
                        yb, yi = divmod(r, RBY)
                        if yi == 0:
                            y_sbs[yb] = ypool.tile([128, RBY, 4, E], f16,
                                                   tag="ysb", name=f"y_sb_{yb}")
                        y_sb = y_sbs[yb]
                        nc.vector.tensor_copy(y_sb[:, yi, 0:2, :], y_ps[:, 0:2, :])
                        nc.scalar.copy(y_sb[:, yi, 2:4, :], y_ps[:, 2:4, :])
                        if yi == RBY - 1:
                            nc.scalar.dma_start(out=y_blk[yb], in_=y_sbs.pop(yb))
                        if og == GO - 1:
                            otsp.pop(ob)

                    # r-pair pipeline: v+out lead, y lags YLAG pairs (the
                    # xbar transpose must land); y rows emitted around
                    # v/out work so single-buffered y PSUM never stalls PE.
                    for pair in range(n_p3 + YLAG):
                        p1 = pair - YLAG
                        if 0 <= p1 < n_p3:
                            stage_y(p1 * 2)
                        if pair < n_p3:
                            stage_v(pair)
                        if 0 <= p1 < n_p3:
                            stage_y(p1 * 2 + 1)
                        if pair < n_p3:
                            stage_out(pair)

    return nc


def _get_program():
    if "nc" not in _CACHE:
        nc = build_program()
        nc.finalize()
        _CACHE["nc"] = nc
    return _CACHE["nc"]


def make_in_maps(x, Wq, Wkv, Wo):
    """Host-side sharding: core = bi*4 + hpi."""
    scale = (64.0 ** -0.5) * (64.0 ** -0.5)
    x = np.asarray(x, np.float32)
    Wq = np.asarray(Wq, np.float32) * np.float32(scale)
    Wkv = np.asarray(Wkv, np.float32)
    Wo = np.asarray(Wo, np.float32)
    b = x.shape[0] // R
    xT = np.ascontiguousarray(
        x.reshape(b, R, N, C).transpose(0, 1, 3, 2)).astype(np.float16)
    in_maps = []
    for core in range(NCORES):
        bi, hpi = divmod(core, 4)
        cols = slice(hpi * HP, (hpi + 1) * HP)
        in_maps.append({
            "xT": xT[bi],
            "wq": np.ascontiguousarray(Wq[:, cols]).astype(np.float16),
            "wk": np.ascontiguousarray(Wkv[:, cols]).astype(np.float16),
            "wv": np.ascontiguousarray(
                Wkv[:, 512 + hpi * HP: 512 + (hpi + 1) * HP]).astype(np.float16),
            "wo": np.ascontiguousarray(Wo[cols, :]).astype(np.float16),
        })
    return in_maps


def combine_outputs(ys, bo):
    """ys: list of 8 [R, N, E] partials in core order; returns [B, n, dim]."""
    y0 = ys[0] + ys[1] + ys[2] + ys[3]
    y1 = ys[4] + ys[5] + ys[6] + ys[7]
    y = np.concatenate([y0, y1], axis=0).reshape(2 * R, N, E)
    return (y + np.asarray(bo, np.float32)).astype(np.float32)


def kernel(x, Wq, Wkv, Wo, bo, tie_attn_dim):
    assert int(tie_attn_dim) == R, f"hardcoded for tie_attn_dim={R}"
    from concourse.bass_utils import run_bass_kernel_spmd

    nc = _get_program()
    in_maps = make_in_maps(x, Wq, Wkv, Wo)
    res = run_bass_kernel_spmd(nc, in_maps, list(range(NCORES)))
    ys = [np.asarray(res.results[c]["y"], np.float32) for c in range(NCORES)]
    return combine_outputs(ys, bo)
